# revision 41
# baseline (speedup 1.0000x reference)
"""Trainium2 Bass kernel for ConvexContractionAttention.

Math notes (derived from the reference):
  block(xi, w, b, a, g, beta) with h = xi*softplus(w)+b, h' = h @ qr(a).Q,
  then batch-norm over (B,T) per (d,j) feature reduces to an affine map of
  the centered input channel:
      out[b,t,d,j] = (xi[b,t,d] - mean_d(xi)) * A[d,j] + beta[d,j]
      A[d,j] = u[d,j]*g[d,j] / sqrt(var_d(xi)*u[d,j]^2 + eps_norm)
      u = softplus(w) @ Q          (bias b cancels through the mean)
  With beta == 0 (true for setup_inputs), per channel d:
      p    = xcq * xck
      s_j  = sigmoid(gamma*Aq_j*Ak_j * p)
      out0 = xcv * (sum_j s_j*Av_j) / (sum_j s_j + eps_w)
  followed by one more batch-affine-norm over (B,T) per channel.

Sharding: channel dim d=1024 split 128-per-core across 8 cores (fully
independent per channel; no collectives). On-chip layout: channels on the
128 SBUF partitions, B*T=8192 on the free axis; the host pre-transposes
each core's shard so every DMA is contiguous.

v2 pipeline (this file):
  - x staged in fp16 (halves HBM traffic; ~0.05% quantization, well under
    the 2e-2 gate), output stored fp16 and cast to fp32 on host.
  - per-body engine split: ScalarE = 3 sigmoids + Square-accum reductions;
    DVE = bn_stats(q), cheap 4x tensor_scalar work, fp32 recip + PSUM
    reads; PE = den/num j-sums (diag matmuls) incl. the eps term;
    GPSIMD = p product and the final out STT.
  - all tile pools are created once with bufs=2 and rotate per body, so
    consecutive bodies pipeline (body i+1's DMAs overlap body i compute).
"""

import sys

if "/opt/trn_rl_repo" not in sys.path:
    sys.path.insert(0, "/opt/trn_rl_repo")

import contextlib

import numpy as np

import concourse.bacc as bacc
import concourse.tile as tile
from concourse import mybir
from concourse import bass_utils

B, T, D = 4, 2048, 1024
BT = B * T
N_CORES = 8
DL = D // N_CORES  # 128 channels per core == SBUF partitions
GAMMA = 5.0
EPS_NORM = 1e-5
EPS_W = 1e-8

F32 = mybir.dt.float32
F16 = mybir.dt.float16
BF16 = mybir.dt.bfloat16
F32R = mybir.dt.float32r
F8 = mybir.dt.float8e4
Act = mybir.ActivationFunctionType
Alu = mybir.AluOpType

FB = 512          # PSUM bank block
HALF = BT // 2    # 4096
QTR = BT // 4     # 2048

# Engine assignment knobs (tuned empirically).
P_ON_GPSIMD = True
OUT_ON_GPSIMD = True
XC_ON_GPSIMD = False


def _emit_rsqrt(nc, pool, v, n, tag):
    """out = 1/sqrt(v) elementwise on a tiny [DL, n] fp32 tile, DVE-only.

    Bit-trick seed + 3 Newton iterations (~1e-7 rel); avoids the ScalarE
    Sqrt table set so the whole kernel stays on the sigmoid set.
    """
    U32 = mybir.dt.uint32
    bitsf = pool.tile([DL, n], F32, name=f"rsq_b_{tag}", tag=f"rsq_b_{tag}")
    nc.vector.tensor_copy(bitsf, v.bitcast(U32))
    nc.vector.tensor_scalar(
        out=bitsf, in0=bitsf, scalar1=-0.5, scalar2=1597463007.0,
        op0=Alu.mult, op1=Alu.add,
    )
    yu = pool.tile([DL, n], U32, name=f"rsq_y_{tag}", tag=f"rsq_y_{tag}")
    nc.vector.tensor_copy(yu, bitsf)
    y = yu.bitcast(F32)
    t = pool.tile([DL, n], F32, name=f"rsq_t_{tag}", tag=f"rsq_t_{tag}")
    for _ in range(3):
        nc.vector.tensor_mul(t, y, y)          # y^2
        nc.vector.tensor_mul(t, t, v)          # v*y^2
        nc.vector.tensor_scalar(
            out=t, in0=t, scalar1=-0.5, scalar2=1.5, op0=Alu.mult, op1=Alu.add,
        )                                      # 1.5 - 0.5*v*y^2
        nc.vector.tensor_mul(y, y, t)
    return y


def _emit_loads(nc, dram, pools, glob_tiles):
    """DMA issues only. Returns context for _emit_stats/_emit_main."""
    res, consts, temps, stage, psum = pools
    ident_sb, epsmat, ones_r = glob_tiles
    chunks = ("q", "k", "v")

    # ---- one packed parameter DMA ----
    pp = consts.tile([DL, 20], F32, name="pp", tag="pp")
    nc.sync.dma_start(out=pp, in_=dram["pp"])
    u_sb = {"q": pp[:, 0:3], "k": pp[:, 6:9], "v": pp[:, 12:15]}
    ug_sb = {"q": pp[:, 3:6], "k": pp[:, 9:12], "v": pp[:, 15:18]}
    g_out_sb = pp[:, 18:19]
    b_out_sb = pp[:, 19:20]

    # ---- x DMAs (fp16 compute copy + fp8 transposed stats copy) ----
    x_sb, xt_sb = {}, {}
    for p in chunks:
        x_sb[p] = res.tile([DL, BT], F16, name=f"x_{p}", tag=f"x_{p}")
        nc.sync.dma_start(out=x_sb[p], in_=dram["x" + p])
        xt_sb[p] = []
        for i in range(2):
            xh = res.tile([DL, HALF], F8, name=f"xt_{p}{i}", tag="xt", bufs=2)
            nc.sync.dma_start(out=xh, in_=dram["xt" + p][:, i * HALF:(i + 1) * HALF])
            xt_sb[p].append(xh)

    return dict(x_sb=x_sb, xt_sb=xt_sb, pp=pp, u_sb=u_sb, ug_sb=ug_sb,
                g_out_sb=g_out_sb, b_out_sb=b_out_sb)


def _emit_stats(nc, dram, pools, glob_tiles, c):
    """Stats + per-body constants (emitted after the previous body's main
    loop so the PE/DVE streams never block on the next body's inputs)."""
    res, consts, temps, stage, psum = pools
    ident_sb, epsmat, ones_r = glob_tiles
    chunks = ("q", "k", "v")
    x_sb, xt_sb = c["x_sb"], c["xt_sb"]
    u_sb, ug_sb = c["u_sb"], c["ug_sb"]

    # sum(x) per chunk via DVE tensor_scalar accum (4x mode)
    sums = {}
    for p in chunks:
        sums[p] = consts.tile([DL, 4], F32, name=f"sum_{p}", tag=f"sum_{p}")
        for h in range(4):
            sl = slice(h * QTR, (h + 1) * QTR)
            scr16 = temps.tile([DL, QTR], F16, name="scp", tag="xcq", bufs=2)
            nc.vector.tensor_scalar(
                out=scr16, in0=x_sb[p][:, sl], scalar1=1.0, scalar2=0.0,
                op0=Alu.mult, op1=Alu.add, accum_out=sums[p][:, h:h + 1],
            )
    # sum(x^2) per chunk via PE Gram on the fp8 transposed copy: G = sum_i
    # Xi^T Xi accumulated in PSUM; diag(G)[d] = sum_t x[d,t]^2.
    sqs = {}
    for p in chunks:
        gfull = psum.tile([DL, QTR], F32, name=f"g_{p}", tag="pden", bufs=1)
        gps = gfull[:, 0:DL]
        for i in range(64):
            xt_i = xt_sb[p][i // 32][:, (i % 32) * DL:(i % 32 + 1) * DL]
            nc.tensor.matmul(out=gps, lhsT=xt_i, rhs=xt_i,
                             start=(i == 0), stop=(i == 63))
        gd = temps.tile([DL, DL], F32, name="gd", tag="gd")
        nc.vector.tensor_mul(gd, gps, ident_sb)
        sqs[p] = consts.tile([DL, 1], F32, name=f"sq_{p}", tag=f"sq_{p}")
        nc.vector.tensor_reduce(sqs[p], gd, axis=mybir.AxisListType.X, op=Alu.add)

    # per-chunk mean/var -> A matrices
    mv = {}
    for p in chunks:
        mvp = consts.tile([DL, 2], F32, name=f"mv_{p}", tag=f"mv_{p}")
        ssum = consts.tile([DL, 1], F32, name=f"ss_{p}", tag=f"ss_{p}")
        nc.vector.tensor_reduce(ssum, sums[p], axis=mybir.AxisListType.X, op=Alu.add)
        nc.vector.tensor_scalar_mul(out=mvp[:, 0:1], in0=ssum, scalar1=1.0 / BT)
        msq = consts.tile([DL, 1], F32, name=f"msq_{p}", tag=f"msq_{p}")
        nc.vector.tensor_mul(msq, mvp[:, 0:1], mvp[:, 0:1])
        nc.vector.scalar_tensor_tensor(
            out=mvp[:, 1:2], in0=sqs[p], scalar=1.0 / BT, in1=msq,
            op0=Alu.mult, op1=Alu.subtract,
        )
        mv[p] = mvp

    vterm = consts.tile([DL, 9], F32, name="vt", tag="vt")
    for pi, p in enumerate(chunks):
        usq = consts.tile([DL, 3], F32, name=f"usq_{p}", tag=f"usq_{p}")
        nc.vector.tensor_mul(usq, u_sb[p], u_sb[p])
        nc.vector.tensor_scalar(
            out=vterm[:, 3 * pi:3 * pi + 3], in0=usq, scalar1=mv[p][:, 1:2],
            scalar2=EPS_NORM, op0=Alu.mult, op1=Alu.add,
        )
    inv9 = _emit_rsqrt(nc, consts, vterm, 9, "A")
    Amat = {}
    for pi, p in enumerate(chunks):
        Amat[p] = consts.tile([DL, 3], F32, name=f"A_{p}", tag=f"A_{p}")
        nc.vector.tensor_mul(Amat[p], ug_sb[p], inv9[:, 3 * pi:3 * pi + 3])
    cmat = consts.tile([DL, 3], F32, name="cmat", tag="cmat")
    nc.vector.tensor_mul(cmat, Amat["q"], Amat["k"])
    nc.vector.tensor_scalar_mul(out=cmat, in0=cmat, scalar1=GAMMA)

    muq = mv["q"][:, 0:1]
    muk = mv["k"][:, 0:1]
    muv = mv["v"][:, 0:1]
    Av = Amat["v"]

    # diag(Av_j) in f32r for the num matmuls (f32r keeps the sigmoid tail —
    # fp16 flushes sigma~1e-8..1e-35 to 0 but the reference's tiny-sigma
    # ratios still produce O(1) weights there)
    dg = []
    for j in range(3):
        d = consts.tile([DL, DL], BF16, name=f"dg{j}", tag=f"dg{j}")
        nc.vector.tensor_scalar_mul(out=d, in0=ident_sb, scalar1=Av[:, j:j + 1])
        dg.append(d)

    c.update(muq=muq, muk=muk, muv=muv, cmat=cmat, dg=dg)
    return c


def _emit_main(nc, dram, pools, glob_tiles, c):
    """Main loop + out-stats + final affine + store for a prepared body."""
    res, consts, temps, stage, psum = pools
    ident_sb, epsmat, ones_r = glob_tiles
    x_sb = c["x_sb"]
    muq, muk, muv = c["muq"], c["muk"], c["muv"]
    cmat, dg = c["cmat"], c["dg"]

    out_sb = res.tile([DL, BT], F16, name="out", tag="out", bufs=2)
    osum = consts.tile([DL, 4], F32, name="osum", tag="osum")
    osq = consts.tile([DL, 4], F32, name="osq", tag="osq")

    # xc/p for all quarters upfront: gives GPSIMD and ScalarE lookahead
    pprods = []
    for qt in range(4):
        qsl = slice(qt * QTR, (qt + 1) * QTR)
        xcq = temps.tile([DL, QTR], F16, name="xcq", tag="xcq", bufs=2)
        xck = temps.tile([DL, QTR], F16, name="xck", tag="xck", bufs=2)
        pprod = temps.tile([DL, QTR], F16, name="pprod", tag="pprod", bufs=2)
        nc.vector.tensor_scalar(
            out=xcq, in0=x_sb["q"][:, qsl], scalar1=muq, scalar2=None,
            op0=Alu.subtract,
        )
        nc.vector.tensor_scalar(
            out=xck, in0=x_sb["k"][:, qsl], scalar1=muk, scalar2=None,
            op0=Alu.subtract,
        )
        eng = nc.gpsimd if P_ON_GPSIMD else nc.vector
        eng.tensor_mul(pprod, xcq, xck)
        pprods.append(pprod)

    for qt in range(4):
        qsl = slice(qt * QTR, (qt + 1) * QTR)
        xcv = temps.tile([DL, QTR], F16, name="xcv", tag="xcv", bufs=2)
        nc.vector.tensor_scalar(
            out=xcv, in0=x_sb["v"][:, qsl], scalar1=muv, scalar2=None,
            op0=Alu.subtract,
        )
        # sigmoids (f32r out for the PE)
        sig = []
        for j in range(3):
            s = temps.tile([DL, QTR], BF16, name=f"s{j}", tag=f"s{j}")
            nc.scalar.activation(s, pprods[qt], Act.Sigmoid,
                                 scale=cmat[:, j:j + 1])
            sig.append(s)
        # one 2048-wide block per quarter: den/num on PE (7 matmuls, each
        # lhsT loaded once), recip+tt on DVE, out product on GPSIMD
        pden = psum.tile([DL, QTR], F32, name="pden", tag="pden", bufs=1)
        pnum = psum.tile([DL, QTR], F32, name="pnum", tag="pnum", bufs=1)
        for b4 in range(QTR // FB):
            ps = slice(b4 * FB, (b4 + 1) * FB)
            nc.tensor.matmul(out=pden[:, ps], lhsT=epsmat, rhs=ones_r[:, ps],
                             start=True, stop=False)
        for j in range(3):
            for b4 in range(QTR // FB):
                ps = slice(b4 * FB, (b4 + 1) * FB)
                nc.tensor.matmul(out=pden[:, ps], lhsT=ident_sb,
                                 rhs=sig[j][:, ps],
                                 start=False, stop=(j == 2))
        for j in range(3):
            for b4 in range(QTR // FB):
                ps = slice(b4 * FB, (b4 + 1) * FB)
                nc.tensor.matmul(out=pnum[:, ps], lhsT=dg[j],
                                 rhs=sig[j][:, ps],
                                 start=(j == 0), stop=(j == 2))
        rr = temps.tile([DL, QTR], F32, name="rr", tag="rr")
        nc.vector.reciprocal_approx_fast(out=rr, in_=pden)
        ttb = temps.tile([DL, QTR], F16, name="ttb", tag="ttb", bufs=1)
        nc.vector.tensor_mul(ttb, pnum, rr)
        nc.gpsimd.tensor_mul(out_sb[:, qsl], xcv, ttb)
    for qt in range(4):
        qsl = slice(qt * QTR, (qt + 1) * QTR)
        scr16 = temps.tile([DL, QTR], F16, name="scp", tag="xcq", bufs=2)
        nc.vector.tensor_scalar(
            out=scr16, in0=out_sb[:, qsl], scalar1=1.0, scalar2=0.0,
            op0=Alu.mult, op1=Alu.add, accum_out=osum[:, qt:qt + 1],
        )
        scrb2 = temps.tile([DL, QTR], BF16, name="scb", tag="scb")
        nc.scalar.activation(scrb2, out_sb[:, qsl], Act.Square,
                             accum_out=osq[:, qt:qt + 1])


    # ---- final norm constants ----
    sum_o = consts.tile([DL, 1], F32, name="sum_o", tag="sum_o")
    nc.vector.tensor_reduce(sum_o, osum, axis=mybir.AxisListType.X, op=Alu.add)
    sq_o = consts.tile([DL, 1], F32, name="sq_o", tag="sq_o")
    nc.vector.tensor_reduce(sq_o, osq, axis=mybir.AxisListType.X, op=Alu.add)
    mean_o = consts.tile([DL, 1], F32, name="mean_o", tag="mean_o")
    nc.vector.tensor_scalar_mul(out=mean_o, in0=sum_o, scalar1=1.0 / BT)
    msq_o = consts.tile([DL, 1], F32, name="msq_o", tag="msq_o")
    nc.vector.tensor_mul(msq_o, mean_o, mean_o)
    var_o = consts.tile([DL, 1], F32, name="var_o", tag="var_o")
    nc.vector.scalar_tensor_tensor(
        out=var_o, in0=sq_o, scalar=1.0 / BT, in1=msq_o,
        op0=Alu.mult, op1=Alu.subtract,
    )
    nc.vector.tensor_scalar_add(out=var_o, in0=var_o, scalar1=EPS_NORM)
    rs_o = _emit_rsqrt(nc, consts, var_o, 1, "o")
    fs = consts.tile([DL, 1], F32, name="fs", tag="fs")
    nc.vector.tensor_mul(fs, c["g_out_sb"], rs_o)
    fbt = consts.tile([DL, 1], F32, name="fbt", tag="fbt")
    nc.vector.tensor_mul(fbt, mean_o, fs)
    fb = consts.tile([DL, 1], F32, name="fb", tag="fb")
    nc.vector.tensor_sub(fb, c["b_out_sb"], fbt)

    # ---- final affine + store (fp16, GPSIMD so DVE is free for the next
    # body's stats) ----
    for i in range(4):
        sl = slice(i * QTR, (i + 1) * QTR)
        stg = temps.tile([DL, QTR], F16, name="stg", tag="xcv", bufs=2)
        if i % 2 == 0:
            nc.vector.tensor_scalar(
                out=stg, in0=out_sb[:, sl], scalar1=fs, scalar2=fb,
                op0=Alu.mult, op1=Alu.add,
            )
        else:
            nc.scalar.activation(stg, out_sb[:, sl], Act.Identity,
                                 bias=fb, scale=fs)
        nc.gpsimd.dma_start(out=dram["out"][:, sl], in_=stg)


def build_program(reps=1, variant="v2"):
    nc = bacc.Bacc("TRN2", num_devices=N_CORES)
    dram = {}
    for p in ("q", "k", "v"):
        dram["x" + p] = nc.dram_tensor("x" + p, [DL, BT], F16, kind="ExternalInput").ap()
        dram["xt" + p] = nc.dram_tensor("xt" + p, [DL, BT], F8, kind="ExternalInput").ap()
    dram["pp"] = nc.dram_tensor("pp", [DL, 20], F32, kind="ExternalInput").ap()
    dram["ident"] = nc.dram_tensor("ident", [DL, DL], BF16, kind="ExternalInput").ap()
    dram["out"] = nc.dram_tensor("out", [DL, BT], F16, kind="ExternalOutput").ap()

    with tile.TileContext(nc) as tc:
        with contextlib.ExitStack() as ctx:
            glob = ctx.enter_context(tc.tile_pool(name="glob", bufs=1))
            res = ctx.enter_context(tc.tile_pool(name="res", bufs=2))
            consts = ctx.enter_context(tc.tile_pool(name="consts", bufs=2))
            temps = ctx.enter_context(tc.tile_pool(name="temps", bufs=1))
            stage = ctx.enter_context(tc.tile_pool(name="stage", bufs=2))
            psum = ctx.enter_context(tc.tile_pool(name="psum", bufs=2, space="PSUM"))

            # global constants, loaded once
            ident_sb = glob.tile([DL, DL], BF16, name="ident", tag="ident")
            nc.sync.dma_start(out=ident_sb, in_=dram["ident"])
            epsmat = glob.tile([DL, DL], BF16, name="epsmat", tag="epsmat")
            nc.vector.memset(epsmat, EPS_W / DL)
            ones_r = glob.tile([DL, QTR], BF16, name="ones_r", tag="ones_r")
            nc.vector.memset(ones_r, 1.0)
            warm = glob.tile([DL, 1], F32, name="warm", tag="warm")
            nc.vector.memset(warm, 0.0)
            nc.scalar.activation(warm, warm, Act.Sigmoid)

            glob_tiles = (ident_sb, epsmat, ones_r)
            pools = (res, consts, temps, stage, psum)
            # software-pipelined emission: body i+1's loads are issued
            # before body i's main loop (DMA overlap), and body i+1's
            # stats/consts are emitted after it (so the PE stream runs
            # main(i) matmuls before Gram(i+1), and no engine blocks on a
            # body tail while independent next-body work waits).
            cur = _emit_loads(nc, dram, pools, glob_tiles)
            cur = _emit_stats(nc, dram, pools, glob_tiles, cur)
            for i in range(reps):
                nxt = None
                if i + 1 < reps:
                    nxt = _emit_loads(nc, dram, pools, glob_tiles)
                _emit_main(nc, dram, pools, glob_tiles, cur)
                if nxt is not None:
                    cur = _emit_stats(nc, dram, pools, glob_tiles, nxt)
    nc.compile()
    return nc


def _softplus(x):
    return np.log1p(np.exp(-np.abs(x))) + np.maximum(x, 0.0)


def _host_params(w, b, a, g, beta):
    """Return (u, u*g) per channel (bias b cancels through the mean)."""
    Q = np.linalg.qr(np.asarray(a, dtype=np.float64))[0].astype(np.float32)
    u = np.einsum("di,dij->dj", _softplus(np.asarray(w, np.float64)).astype(np.float32), Q)
    return u, u * np.asarray(g, np.float32)


def _reference_fallback(x, wq, bq, aq, gq, betaq, wk, bk, ak, gk, betak,
                        wv, bv, av, gv, betav, g_out, b_out):
    """General-path numpy fallback (only used if some beta is nonzero)."""
    def block(xi, w, b, a, g, beta):
        h = xi[..., None] * _softplus(w) + b
        Q = np.linalg.qr(a)[0]
        h = np.einsum("btdi,dij->btdj", h, Q)
        mean = h.mean(axis=(0, 1))
        var = h.var(axis=(0, 1))
        return (h - mean) / np.sqrt(var + EPS_NORM) * g + beta

    d = D
    Qp = block(x[..., :d], wq, bq, aq, gq, betaq)
    Kp = block(x[..., d:2 * d], wk, bk, ak, gk, betak)
    Vp = block(x[..., 2 * d:], wv, bv, av, gv, betav)
    scores = 1.0 / (1.0 + np.exp(-GAMMA * (Qp * Kp)))
    weights = scores / (scores.sum(axis=-1, keepdims=True) + EPS_W)
    out = (weights * Vp).sum(axis=-1)
    mean = out.mean(axis=(0, 1))
    var = out.var(axis=(0, 1))
    return ((out - mean) / np.sqrt(var + EPS_NORM) * g_out + b_out).astype(np.float32)


_NC_CACHE = {}

VARIANT = "v2"


def _get_program(reps=1, variant=None):
    if variant is None:
        variant = VARIANT
    key = (reps, variant)
    if key not in _NC_CACHE:
        _NC_CACHE[key] = build_program(reps, variant)
    return _NC_CACHE[key]


def _make_in_maps(x, params):
    """params: dict p -> (u, ug) full (D,3); x: (B,T,3D). Returns per-core maps."""
    x2 = np.asarray(x, np.float32).reshape(BT, 3 * D)
    # one-pass transpose into (24 blocks, DL channels, BT) channel-major, fp16
    xt = np.ascontiguousarray(
        x2.reshape(BT, 3 * N_CORES, DL).transpose(1, 2, 0)).astype(np.float16)
    in_maps = []
    for c in range(N_CORES):
        m = {}
        pp = np.empty((DL, 20), np.float32)
        import ml_dtypes
        for pi, p in enumerate(("q", "k", "v")):
            xc = xt[pi * N_CORES + c]
            m["x" + p] = xc
            m["xt" + p] = np.ascontiguousarray(
                xc.reshape(DL, BT // DL, DL).transpose(2, 1, 0).reshape(DL, BT)
            ).astype(ml_dtypes.float8_e4m3)
            u, ug = params[p]
            pp[:, 6 * pi:6 * pi + 3] = u[c * DL:(c + 1) * DL]
            pp[:, 6 * pi + 3:6 * pi + 6] = ug[c * DL:(c + 1) * DL]
        pp[:, 18] = params["g_out"][c * DL:(c + 1) * DL]
        pp[:, 19] = params["b_out"][c * DL:(c + 1) * DL]
        m["pp"] = pp
        import ml_dtypes
        m["ident"] = np.eye(DL, dtype=ml_dtypes.bfloat16)
        in_maps.append(m)
    return in_maps


def kernel(x, wq, bq, aq, gq, betaq, wk, bk, ak, gk, betak,
           wv, bv, av, gv, betav, g_out, b_out):
    if (np.any(np.asarray(betaq)) or np.any(np.asarray(betak))
            or np.any(np.asarray(betav))):
        return _reference_fallback(x, wq, bq, aq, gq, betaq, wk, bk, ak, gk,
                                   betak, wv, bv, av, gv, betav, g_out, b_out)

    params = {
        "q": _host_params(wq, bq, aq, gq, betaq),
        "k": _host_params(wk, bk, ak, gk, betak),
        "v": _host_params(wv, bv, av, gv, betav),
        "g_out": np.asarray(g_out, np.float32),
        "b_out": np.asarray(b_out, np.float32),
    }
    nc = _get_program()
    in_maps = _make_in_maps(x, params)
    try:
        per_core = _run_cached(nc, in_maps)
    except Exception:
        res = bass_utils.run_bass_kernel_spmd(
            nc, in_maps, core_ids=list(range(N_CORES)))
        per_core = [res.results[c]["out"] for c in range(N_CORES)]
    out = np.empty((BT, D), np.float32)
    for c in range(N_CORES):
        out[:, c * DL:(c + 1) * DL] = np.asarray(per_core[c], np.float32).T
    return out.reshape(B, T, D)


_RUNNER_CACHE = {}


def _run_cached(nc, in_maps):
    """Jit the bass_exec shard_map once; later kernel() calls only restage
    inputs (saves ~1-2 s of retracing/recompiling per call)."""
    key = id(nc)
    if key not in _RUNNER_CACHE:
        import jax
        from jax.sharding import Mesh, PartitionSpec, NamedSharding
        try:
            from jax import shard_map
        except ImportError:
            from jax.experimental.shard_map import shard_map
        from concourse import mybir as _mb
        from concourse.bass2jax import (
            _bass_exec_p, install_neuronx_cc_hook, partition_id_tensor)

        install_neuronx_cc_hook()
        pname = nc.partition_id_tensor.name if nc.partition_id_tensor else None
        in_names, out_names, out_avals, zero_outs = [], [], [], []
        for alloc in nc.m.functions[0].allocations:
            if not isinstance(alloc, _mb.MemoryLocationSet):
                continue
            name = alloc.memorylocations[0].name
            if alloc.kind == "ExternalInput":
                if name != pname:
                    in_names.append(name)
            elif alloc.kind == "ExternalOutput":
                out_names.append(name)
                shp = tuple(alloc.tensor_shape)
                dt_np = _mb.dt.np(alloc.dtype)
                out_avals.append(jax.core.ShapedArray(shp, dt_np))
                zero_outs.append(np.zeros(shp, dt_np))
        all_in = list(in_names) + list(out_names)
        if pname is not None:
            all_in.append(pname)

        def _body(*args):
            operands = list(args)
            if pname is not None:
                operands.append(partition_id_tensor())
            return tuple(_bass_exec_p.bind(
                *operands, out_avals=tuple(out_avals), in_names=tuple(all_in),
                out_names=tuple(out_names), lowering_input_output_aliases=(),
                sim_require_finite=True, sim_require_nnan=True, nc=nc))

        devices = jax.devices()[:N_CORES]
        mesh = Mesh(np.asarray(devices), ("core",))
        nspec = (PartitionSpec("core"),) * (len(in_names) + len(out_names))
        try:
            smapped = shard_map(_body, mesh=mesh, in_specs=nspec,
                                out_specs=(PartitionSpec("core"),) * len(out_names),
                                check_vma=False)
        except TypeError:
            smapped = shard_map(_body, mesh=mesh, in_specs=nspec,
                                out_specs=(PartitionSpec("core"),) * len(out_names),
                                check_rep=False)
        jitted = jax.jit(smapped, keep_unused=True)
        sh = NamedSharding(mesh, PartitionSpec("core"))
        zconcat = [
            jax.device_put(
                np.zeros((N_CORES * z.shape[0], *z.shape[1:]), z.dtype), sh)
            for z in zero_outs]
        _RUNNER_CACHE[key] = (jitted, in_names, out_names, out_avals, sh, zconcat)
    import jax
    jitted, in_names, out_names, out_avals, sh, zconcat = _RUNNER_CACHE[key]
    args = [
        jax.device_put(
            np.concatenate([in_maps[c][nm] for c in range(N_CORES)], axis=0), sh)
        for nm in in_names]
    outs = jitted(*args, *zconcat)
    oi = out_names.index("out")
    full = np.asarray(outs[oi]).reshape(N_CORES, *out_avals[oi].shape)
    return [full[c] for c in range(N_CORES)]


# revision 43
# speedup vs baseline: 1.7243x; 1.7243x over previous
"""Trainium2 Bass kernel for ConvexContractionAttention.

Math notes (derived from the reference):
  block(xi, w, b, a, g, beta) with h = xi*softplus(w)+b, h' = h @ qr(a).Q,
  then batch-norm over (B,T) per (d,j) feature reduces to an affine map of
  the centered input channel:
      out[b,t,d,j] = (xi[b,t,d] - mean_d(xi)) * A[d,j] + beta[d,j]
      A[d,j] = u[d,j]*g[d,j] / sqrt(var_d(xi)*u[d,j]^2 + eps_norm)
      u = softplus(w) @ Q          (bias b cancels through the mean)
  With beta == 0 (true for setup_inputs), per channel d:
      p    = xcq * xck
      s_j  = sigmoid(gamma*Aq_j*Ak_j * p)
      out0 = xcv * (sum_j s_j*Av_j) / (sum_j s_j + eps_w)
  followed by one more batch-affine-norm over (B,T) per channel.

Sharding: channel dim d=1024 split 128-per-core across 8 cores (fully
independent per channel; no collectives). On-chip layout: channels on the
128 SBUF partitions, B*T=8192 on the free axis; the host pre-transposes
each core's shard so every DMA is contiguous.

v2 pipeline (this file):
  - x staged in fp16 (halves HBM traffic; ~0.05% quantization, well under
    the 2e-2 gate), output stored fp16 and cast to fp32 on host.
  - per-body engine split: ScalarE = 3 sigmoids + Square-accum reductions;
    DVE = bn_stats(q), cheap 4x tensor_scalar work, fp32 recip + PSUM
    reads; PE = den/num j-sums (diag matmuls) incl. the eps term;
    GPSIMD = p product and the final out STT.
  - all tile pools are created once with bufs=2 and rotate per body, so
    consecutive bodies pipeline (body i+1's DMAs overlap body i compute).
"""

import sys

if "/opt/trn_rl_repo" not in sys.path:
    sys.path.insert(0, "/opt/trn_rl_repo")

import contextlib

import numpy as np

import concourse.bacc as bacc
import concourse.tile as tile
from concourse import mybir
from concourse import bass_utils

B, T, D = 4, 2048, 1024
BT = B * T
N_CORES = 8
DL = D // N_CORES  # 128 channels per core == SBUF partitions
GAMMA = 5.0
EPS_NORM = 1e-5
EPS_W = 1e-8

F32 = mybir.dt.float32
F16 = mybir.dt.float16
BF16 = mybir.dt.bfloat16
F32R = mybir.dt.float32r
F8 = mybir.dt.float8e4
Act = mybir.ActivationFunctionType
Alu = mybir.AluOpType

FB = 512          # PSUM bank block
HALF = BT // 2    # 4096
QTR = BT // 4     # 2048

# Engine assignment knobs (tuned empirically).
P_ON_GPSIMD = False
OUT_ON_GPSIMD = False
XC_ON_GPSIMD = False


def _emit_rsqrt(nc, pool, v, n, tag):
    """out = 1/sqrt(v) elementwise on a tiny [DL, n] fp32 tile, DVE-only.

    Bit-trick seed + 3 Newton iterations (~1e-7 rel); avoids the ScalarE
    Sqrt table set so the whole kernel stays on the sigmoid set.
    """
    U32 = mybir.dt.uint32
    bitsf = pool.tile([DL, n], F32, name=f"rsq_b_{tag}", tag=f"rsq_b_{tag}")
    nc.vector.tensor_copy(bitsf, v.bitcast(U32))
    nc.vector.tensor_scalar(
        out=bitsf, in0=bitsf, scalar1=-0.5, scalar2=1597463007.0,
        op0=Alu.mult, op1=Alu.add,
    )
    yu = pool.tile([DL, n], U32, name=f"rsq_y_{tag}", tag=f"rsq_y_{tag}")
    nc.vector.tensor_copy(yu, bitsf)
    y = yu.bitcast(F32)
    t = pool.tile([DL, n], F32, name=f"rsq_t_{tag}", tag=f"rsq_t_{tag}")
    for _ in range(3):
        nc.vector.tensor_mul(t, y, y)          # y^2
        nc.vector.tensor_mul(t, t, v)          # v*y^2
        nc.vector.tensor_scalar(
            out=t, in0=t, scalar1=-0.5, scalar2=1.5, op0=Alu.mult, op1=Alu.add,
        )                                      # 1.5 - 0.5*v*y^2
        nc.vector.tensor_mul(y, y, t)
    return y


def _emit_loads(nc, dram, pools, glob_tiles):
    """DMA issues only. Returns context for _emit_stats/_emit_main."""
    res, consts, temps, stage, psum = pools
    ident_sb, epsmat, ones_r = glob_tiles
    chunks = ("q", "k", "v")

    # ---- one packed parameter DMA ----
    pp = consts.tile([DL, 20], F32, name="pp", tag="pp")
    nc.sync.dma_start(out=pp, in_=dram["pp"])
    u_sb = {"q": pp[:, 0:3], "k": pp[:, 6:9], "v": pp[:, 12:15]}
    ug_sb = {"q": pp[:, 3:6], "k": pp[:, 9:12], "v": pp[:, 15:18]}
    g_out_sb = pp[:, 18:19]
    b_out_sb = pp[:, 19:20]

    # ---- x DMAs (fp16 compute copy + fp8 transposed stats copy) ----
    x_sb, xt_sb = {}, {}
    for p in chunks:
        x_sb[p] = res.tile([DL, BT], F16, name=f"x_{p}", tag=f"x_{p}")
        nc.sync.dma_start(out=x_sb[p], in_=dram["x" + p])
        xt_sb[p] = []
        for i in range(2):
            xh = res.tile([DL, HALF], F8, name=f"xt_{p}{i}", tag="xt", bufs=2)
            nc.sync.dma_start(out=xh, in_=dram["xt" + p][:, i * HALF:(i + 1) * HALF])
            xt_sb[p].append(xh)

    return dict(x_sb=x_sb, xt_sb=xt_sb, pp=pp, u_sb=u_sb, ug_sb=ug_sb,
                g_out_sb=g_out_sb, b_out_sb=b_out_sb)


def _emit_stats(nc, dram, pools, glob_tiles, c):
    """Stats + per-body constants (emitted after the previous body's main
    loop so the PE/DVE streams never block on the next body's inputs)."""
    res, consts, temps, stage, psum = pools
    ident_sb, epsmat, ones_r = glob_tiles
    chunks = ("q", "k", "v")
    x_sb, xt_sb = c["x_sb"], c["xt_sb"]
    u_sb, ug_sb = c["u_sb"], c["ug_sb"]

    # sum(x) per chunk via DVE tensor_scalar accum (4x mode)
    sums = {}
    for p in chunks:
        sums[p] = consts.tile([DL, 4], F32, name=f"sum_{p}", tag=f"sum_{p}")
        for h in range(4):
            sl = slice(h * QTR, (h + 1) * QTR)
            scr16 = temps.tile([DL, QTR], F16, name="scp", tag="xcq", bufs=2)
            nc.vector.tensor_scalar(
                out=scr16, in0=x_sb[p][:, sl], scalar1=1.0, scalar2=0.0,
                op0=Alu.mult, op1=Alu.add, accum_out=sums[p][:, h:h + 1],
            )
    # sum(x^2) per chunk via PE Gram on the fp8 transposed copy: G = sum_i
    # Xi^T Xi accumulated in PSUM; diag(G)[d] = sum_t x[d,t]^2.
    sqs = {}
    for p in chunks:
        gfull = psum.tile([DL, QTR], F32, name=f"g_{p}", tag="pden", bufs=1)
        gps = gfull[:, 0:DL]
        for i in range(64):
            xt_i = xt_sb[p][i // 32][:, (i % 32) * DL:(i % 32 + 1) * DL]
            nc.tensor.matmul(out=gps, lhsT=xt_i, rhs=xt_i,
                             start=(i == 0), stop=(i == 63))
        gd = temps.tile([DL, DL], F32, name="gd", tag="gd")
        nc.vector.tensor_mul(gd, gps, ident_sb)
        sqs[p] = consts.tile([DL, 1], F32, name=f"sq_{p}", tag=f"sq_{p}")
        nc.vector.tensor_reduce(sqs[p], gd, axis=mybir.AxisListType.X, op=Alu.add)

    # per-chunk mean/var -> A matrices
    mv = {}
    for p in chunks:
        mvp = consts.tile([DL, 2], F32, name=f"mv_{p}", tag=f"mv_{p}")
        ssum = consts.tile([DL, 1], F32, name=f"ss_{p}", tag=f"ss_{p}")
        nc.vector.tensor_reduce(ssum, sums[p], axis=mybir.AxisListType.X, op=Alu.add)
        nc.vector.tensor_scalar_mul(out=mvp[:, 0:1], in0=ssum, scalar1=1.0 / BT)
        msq = consts.tile([DL, 1], F32, name=f"msq_{p}", tag=f"msq_{p}")
        nc.vector.tensor_mul(msq, mvp[:, 0:1], mvp[:, 0:1])
        nc.vector.scalar_tensor_tensor(
            out=mvp[:, 1:2], in0=sqs[p], scalar=1.0 / BT, in1=msq,
            op0=Alu.mult, op1=Alu.subtract,
        )
        mv[p] = mvp

    vterm = consts.tile([DL, 9], F32, name="vt", tag="vt")
    for pi, p in enumerate(chunks):
        usq = consts.tile([DL, 3], F32, name=f"usq_{p}", tag=f"usq_{p}")
        nc.vector.tensor_mul(usq, u_sb[p], u_sb[p])
        nc.vector.tensor_scalar(
            out=vterm[:, 3 * pi:3 * pi + 3], in0=usq, scalar1=mv[p][:, 1:2],
            scalar2=EPS_NORM, op0=Alu.mult, op1=Alu.add,
        )
    inv9 = _emit_rsqrt(nc, consts, vterm, 9, "A")
    Amat = {}
    for pi, p in enumerate(chunks):
        Amat[p] = consts.tile([DL, 3], F32, name=f"A_{p}", tag=f"A_{p}")
        nc.vector.tensor_mul(Amat[p], ug_sb[p], inv9[:, 3 * pi:3 * pi + 3])
    cmat = consts.tile([DL, 3], F32, name="cmat", tag="cmat")
    nc.vector.tensor_mul(cmat, Amat["q"], Amat["k"])
    nc.vector.tensor_scalar_mul(out=cmat, in0=cmat, scalar1=GAMMA)

    muq = mv["q"][:, 0:1]
    muk = mv["k"][:, 0:1]
    muv = mv["v"][:, 0:1]
    Av = Amat["v"]

    # diag(Av_j) in f32r for the num matmuls (f32r keeps the sigmoid tail —
    # fp16 flushes sigma~1e-8..1e-35 to 0 but the reference's tiny-sigma
    # ratios still produce O(1) weights there)
    dg = []
    for j in range(3):
        d = consts.tile([DL, DL], BF16, name=f"dg{j}", tag=f"dg{j}")
        nc.vector.tensor_scalar_mul(out=d, in0=ident_sb, scalar1=Av[:, j:j + 1])
        dg.append(d)

    c.update(muq=muq, muk=muk, muv=muv, cmat=cmat, dg=dg)
    return c


def _emit_main(nc, dram, pools, glob_tiles, c):
    """Main loop + out-stats + final affine + store for a prepared body."""
    res, consts, temps, stage, psum = pools
    ident_sb, epsmat, ones_r = glob_tiles
    x_sb = c["x_sb"]
    muq, muk, muv = c["muq"], c["muk"], c["muv"]
    cmat, dg = c["cmat"], c["dg"]

    out_sb = res.tile([DL, BT], F16, name="out", tag="out", bufs=2)
    osum = consts.tile([DL, 4], F32, name="osum", tag="osum")
    osq = consts.tile([DL, 4], F32, name="osq", tag="osq")

    # xc/p for all quarters upfront: gives GPSIMD and ScalarE lookahead
    pprods = []
    for qt in range(4):
        qsl = slice(qt * QTR, (qt + 1) * QTR)
        xcq = temps.tile([DL, QTR], F16, name="xcq", tag="xcq", bufs=2)
        xck = temps.tile([DL, QTR], F16, name="xck", tag="xck", bufs=2)
        pprod = temps.tile([DL, QTR], F16, name="pprod", tag="pprod", bufs=2)
        nc.vector.tensor_scalar(
            out=xcq, in0=x_sb["q"][:, qsl], scalar1=muq, scalar2=None,
            op0=Alu.subtract,
        )
        nc.vector.tensor_scalar(
            out=xck, in0=x_sb["k"][:, qsl], scalar1=muk, scalar2=None,
            op0=Alu.subtract,
        )
        eng = nc.gpsimd if P_ON_GPSIMD else nc.vector
        eng.tensor_mul(pprod, xcq, xck)  # fp16 TT: DVE 2x or Pool
        pprods.append(pprod)

    for qt in range(4):
        qsl = slice(qt * QTR, (qt + 1) * QTR)
        # sigmoids (bf16 out for the PE)
        sig = []
        for j in range(3):
            s = temps.tile([DL, QTR], BF16, name=f"s{j}", tag=f"s{j}")
            nc.scalar.activation(s, pprods[qt], Act.Sigmoid,
                                 scale=cmat[:, j:j + 1])
            sig.append(s)
        # one 2048-wide block per quarter: den/num on PE (7 matmuls, each
        # lhsT loaded once), recip+tt on DVE, out product on GPSIMD
        pden = psum.tile([DL, QTR], F32, name="pden", tag="pden", bufs=1)
        pnum = psum.tile([DL, QTR], F32, name="pnum", tag="pnum", bufs=1)
        for b4 in range(QTR // FB):
            ps = slice(b4 * FB, (b4 + 1) * FB)
            nc.tensor.matmul(out=pden[:, ps], lhsT=epsmat, rhs=ones_r[:, ps],
                             start=True, stop=False)
        for j in range(3):
            for b4 in range(QTR // FB):
                ps = slice(b4 * FB, (b4 + 1) * FB)
                nc.tensor.matmul(out=pden[:, ps], lhsT=ident_sb,
                                 rhs=sig[j][:, ps],
                                 start=False, stop=(j == 2))
        for j in range(3):
            for b4 in range(QTR // FB):
                ps = slice(b4 * FB, (b4 + 1) * FB)
                nc.tensor.matmul(out=pnum[:, ps], lhsT=dg[j],
                                 rhs=sig[j][:, ps],
                                 start=(j == 0), stop=(j == 2))
        rr = temps.tile([DL, QTR], F32, name="rr", tag="rr")
        nc.vector.reciprocal_approx_fast(out=rr, in_=pden)
        ttb = temps.tile([DL, QTR], F16, name="ttb", tag="ttb", bufs=1)
        nc.vector.tensor_mul(ttb, pnum, rr)
        nc.vector.scalar_tensor_tensor(
            out=out_sb[:, qsl], in0=x_sb["v"][:, qsl], scalar=muv, in1=ttb,
            op0=Alu.subtract, op1=Alu.mult, accum_out=osum[:, qt:qt + 1],
        )
    for qt in range(4):
        qsl = slice(qt * QTR, (qt + 1) * QTR)
        scrb2 = temps.tile([DL, QTR], BF16, name="scb", tag="scb")
        nc.scalar.activation(scrb2, out_sb[:, qsl], Act.Square,
                             accum_out=osq[:, qt:qt + 1])


    # ---- final norm constants ----
    sum_o = consts.tile([DL, 1], F32, name="sum_o", tag="sum_o")
    nc.vector.tensor_reduce(sum_o, osum, axis=mybir.AxisListType.X, op=Alu.add)
    sq_o = consts.tile([DL, 1], F32, name="sq_o", tag="sq_o")
    nc.vector.tensor_reduce(sq_o, osq, axis=mybir.AxisListType.X, op=Alu.add)
    mean_o = consts.tile([DL, 1], F32, name="mean_o", tag="mean_o")
    nc.vector.tensor_scalar_mul(out=mean_o, in0=sum_o, scalar1=1.0 / BT)
    msq_o = consts.tile([DL, 1], F32, name="msq_o", tag="msq_o")
    nc.vector.tensor_mul(msq_o, mean_o, mean_o)
    var_o = consts.tile([DL, 1], F32, name="var_o", tag="var_o")
    nc.vector.scalar_tensor_tensor(
        out=var_o, in0=sq_o, scalar=1.0 / BT, in1=msq_o,
        op0=Alu.mult, op1=Alu.subtract,
    )
    nc.vector.tensor_scalar_add(out=var_o, in0=var_o, scalar1=EPS_NORM)
    rs_o = _emit_rsqrt(nc, consts, var_o, 1, "o")
    fs = consts.tile([DL, 1], F32, name="fs", tag="fs")
    nc.vector.tensor_mul(fs, c["g_out_sb"], rs_o)
    fbt = consts.tile([DL, 1], F32, name="fbt", tag="fbt")
    nc.vector.tensor_mul(fbt, mean_o, fs)
    fb = consts.tile([DL, 1], F32, name="fb", tag="fb")
    nc.vector.tensor_sub(fb, c["b_out_sb"], fbt)

    # ---- final affine + store (fp16, GPSIMD so DVE is free for the next
    # body's stats) ----
    for i in range(4):
        sl = slice(i * QTR, (i + 1) * QTR)
        stg = temps.tile([DL, QTR], F16, name="stg", tag="xcv", bufs=2)
        if i % 2 == 0:
            nc.vector.tensor_scalar(
                out=stg, in0=out_sb[:, sl], scalar1=fs, scalar2=fb,
                op0=Alu.mult, op1=Alu.add,
            )
        else:
            nc.scalar.activation(stg, out_sb[:, sl], Act.Identity,
                                 bias=fb, scale=fs)
        nc.gpsimd.dma_start(out=dram["out"][:, sl], in_=stg)


def build_program(reps=1, variant="v2"):
    nc = bacc.Bacc("TRN2", num_devices=N_CORES)
    dram = {}
    for p in ("q", "k", "v"):
        dram["x" + p] = nc.dram_tensor("x" + p, [DL, BT], F16, kind="ExternalInput").ap()
        dram["xt" + p] = nc.dram_tensor("xt" + p, [DL, BT], F8, kind="ExternalInput").ap()
    dram["pp"] = nc.dram_tensor("pp", [DL, 20], F32, kind="ExternalInput").ap()
    dram["ident"] = nc.dram_tensor("ident", [DL, DL], BF16, kind="ExternalInput").ap()
    dram["out"] = nc.dram_tensor("out", [DL, BT], F16, kind="ExternalOutput").ap()

    with tile.TileContext(nc) as tc:
        with contextlib.ExitStack() as ctx:
            glob = ctx.enter_context(tc.tile_pool(name="glob", bufs=1))
            res = ctx.enter_context(tc.tile_pool(name="res", bufs=2))
            consts = ctx.enter_context(tc.tile_pool(name="consts", bufs=2))
            temps = ctx.enter_context(tc.tile_pool(name="temps", bufs=1))
            stage = ctx.enter_context(tc.tile_pool(name="stage", bufs=2))
            psum = ctx.enter_context(tc.tile_pool(name="psum", bufs=2, space="PSUM"))

            # global constants, loaded once
            ident_sb = glob.tile([DL, DL], BF16, name="ident", tag="ident")
            nc.sync.dma_start(out=ident_sb, in_=dram["ident"])
            epsmat = glob.tile([DL, DL], BF16, name="epsmat", tag="epsmat")
            nc.vector.memset(epsmat, EPS_W / DL)
            ones_r = glob.tile([DL, QTR], BF16, name="ones_r", tag="ones_r")
            nc.vector.memset(ones_r, 1.0)
            warm = glob.tile([DL, 1], F32, name="warm", tag="warm")
            nc.vector.memset(warm, 0.0)
            nc.scalar.activation(warm, warm, Act.Sigmoid)

            glob_tiles = (ident_sb, epsmat, ones_r)
            pools = (res, consts, temps, stage, psum)
            # software-pipelined emission: body i+1's loads are issued
            # before body i's main loop (DMA overlap), and body i+1's
            # stats/consts are emitted after it (so the PE stream runs
            # main(i) matmuls before Gram(i+1), and no engine blocks on a
            # body tail while independent next-body work waits).
            cur = _emit_loads(nc, dram, pools, glob_tiles)
            cur = _emit_stats(nc, dram, pools, glob_tiles, cur)
            for i in range(reps):
                nxt = None
                if i + 1 < reps:
                    nxt = _emit_loads(nc, dram, pools, glob_tiles)
                _emit_main(nc, dram, pools, glob_tiles, cur)
                if nxt is not None:
                    cur = _emit_stats(nc, dram, pools, glob_tiles, nxt)
    nc.compile()
    return nc


def _softplus(x):
    return np.log1p(np.exp(-np.abs(x))) + np.maximum(x, 0.0)


def _host_params(w, b, a, g, beta):
    """Return (u, u*g) per channel (bias b cancels through the mean)."""
    Q = np.linalg.qr(np.asarray(a, dtype=np.float64))[0].astype(np.float32)
    u = np.einsum("di,dij->dj", _softplus(np.asarray(w, np.float64)).astype(np.float32), Q)
    return u, u * np.asarray(g, np.float32)


def _reference_fallback(x, wq, bq, aq, gq, betaq, wk, bk, ak, gk, betak,
                        wv, bv, av, gv, betav, g_out, b_out):
    """General-path numpy fallback (only used if some beta is nonzero)."""
    def block(xi, w, b, a, g, beta):
        h = xi[..., None] * _softplus(w) + b
        Q = np.linalg.qr(a)[0]
        h = np.einsum("btdi,dij->btdj", h, Q)
        mean = h.mean(axis=(0, 1))
        var = h.var(axis=(0, 1))
        return (h - mean) / np.sqrt(var + EPS_NORM) * g + beta

    d = D
    Qp = block(x[..., :d], wq, bq, aq, gq, betaq)
    Kp = block(x[..., d:2 * d], wk, bk, ak, gk, betak)
    Vp = block(x[..., 2 * d:], wv, bv, av, gv, betav)
    scores = 1.0 / (1.0 + np.exp(-GAMMA * (Qp * Kp)))
    weights = scores / (scores.sum(axis=-1, keepdims=True) + EPS_W)
    out = (weights * Vp).sum(axis=-1)
    mean = out.mean(axis=(0, 1))
    var = out.var(axis=(0, 1))
    return ((out - mean) / np.sqrt(var + EPS_NORM) * g_out + b_out).astype(np.float32)


_NC_CACHE = {}

VARIANT = "v2"


def _get_program(reps=1, variant=None):
    if variant is None:
        variant = VARIANT
    key = (reps, variant)
    if key not in _NC_CACHE:
        _NC_CACHE[key] = build_program(reps, variant)
    return _NC_CACHE[key]


def _make_in_maps(x, params):
    """params: dict p -> (u, ug) full (D,3); x: (B,T,3D). Returns per-core maps."""
    x2 = np.asarray(x, np.float32).reshape(BT, 3 * D)
    # one-pass transpose into (24 blocks, DL channels, BT) channel-major, fp16
    xt = np.ascontiguousarray(
        x2.reshape(BT, 3 * N_CORES, DL).transpose(1, 2, 0)).astype(np.float16)
    in_maps = []
    for c in range(N_CORES):
        m = {}
        pp = np.empty((DL, 20), np.float32)
        import ml_dtypes
        for pi, p in enumerate(("q", "k", "v")):
            xc = xt[pi * N_CORES + c]
            m["x" + p] = xc
            m["xt" + p] = np.ascontiguousarray(
                xc.reshape(DL, BT // DL, DL).transpose(2, 1, 0).reshape(DL, BT)
            ).astype(ml_dtypes.float8_e4m3)
            u, ug = params[p]
            pp[:, 6 * pi:6 * pi + 3] = u[c * DL:(c + 1) * DL]
            pp[:, 6 * pi + 3:6 * pi + 6] = ug[c * DL:(c + 1) * DL]
        pp[:, 18] = params["g_out"][c * DL:(c + 1) * DL]
        pp[:, 19] = params["b_out"][c * DL:(c + 1) * DL]
        m["pp"] = pp
        import ml_dtypes
        m["ident"] = np.eye(DL, dtype=ml_dtypes.bfloat16)
        in_maps.append(m)
    return in_maps


def kernel(x, wq, bq, aq, gq, betaq, wk, bk, ak, gk, betak,
           wv, bv, av, gv, betav, g_out, b_out):
    if (np.any(np.asarray(betaq)) or np.any(np.asarray(betak))
            or np.any(np.asarray(betav))):
        return _reference_fallback(x, wq, bq, aq, gq, betaq, wk, bk, ak, gk,
                                   betak, wv, bv, av, gv, betav, g_out, b_out)

    params = {
        "q": _host_params(wq, bq, aq, gq, betaq),
        "k": _host_params(wk, bk, ak, gk, betak),
        "v": _host_params(wv, bv, av, gv, betav),
        "g_out": np.asarray(g_out, np.float32),
        "b_out": np.asarray(b_out, np.float32),
    }
    nc = _get_program()
    in_maps = _make_in_maps(x, params)
    try:
        per_core = _run_cached(nc, in_maps)
    except Exception:
        res = bass_utils.run_bass_kernel_spmd(
            nc, in_maps, core_ids=list(range(N_CORES)))
        per_core = [res.results[c]["out"] for c in range(N_CORES)]
    out = np.empty((BT, D), np.float32)
    for c in range(N_CORES):
        out[:, c * DL:(c + 1) * DL] = np.asarray(per_core[c], np.float32).T
    return out.reshape(B, T, D)


_RUNNER_CACHE = {}


def _run_cached(nc, in_maps):
    """Jit the bass_exec shard_map once; later kernel() calls only restage
    inputs (saves ~1-2 s of retracing/recompiling per call)."""
    key = id(nc)
    if key not in _RUNNER_CACHE:
        import jax
        from jax.sharding import Mesh, PartitionSpec, NamedSharding
        try:
            from jax import shard_map
        except ImportError:
            from jax.experimental.shard_map import shard_map
        from concourse import mybir as _mb
        from concourse.bass2jax import (
            _bass_exec_p, install_neuronx_cc_hook, partition_id_tensor)

        install_neuronx_cc_hook()
        pname = nc.partition_id_tensor.name if nc.partition_id_tensor else None
        in_names, out_names, out_avals, zero_outs = [], [], [], []
        for alloc in nc.m.functions[0].allocations:
            if not isinstance(alloc, _mb.MemoryLocationSet):
                continue
            name = alloc.memorylocations[0].name
            if alloc.kind == "ExternalInput":
                if name != pname:
                    in_names.append(name)
            elif alloc.kind == "ExternalOutput":
                out_names.append(name)
                shp = tuple(alloc.tensor_shape)
                dt_np = _mb.dt.np(alloc.dtype)
                out_avals.append(jax.core.ShapedArray(shp, dt_np))
                zero_outs.append(np.zeros(shp, dt_np))
        all_in = list(in_names) + list(out_names)
        if pname is not None:
            all_in.append(pname)

        def _body(*args):
            operands = list(args)
            if pname is not None:
                operands.append(partition_id_tensor())
            return tuple(_bass_exec_p.bind(
                *operands, out_avals=tuple(out_avals), in_names=tuple(all_in),
                out_names=tuple(out_names), lowering_input_output_aliases=(),
                sim_require_finite=True, sim_require_nnan=True, nc=nc))

        devices = jax.devices()[:N_CORES]
        mesh = Mesh(np.asarray(devices), ("core",))
        nspec = (PartitionSpec("core"),) * (len(in_names) + len(out_names))
        try:
            smapped = shard_map(_body, mesh=mesh, in_specs=nspec,
                                out_specs=(PartitionSpec("core"),) * len(out_names),
                                check_vma=False)
        except TypeError:
            smapped = shard_map(_body, mesh=mesh, in_specs=nspec,
                                out_specs=(PartitionSpec("core"),) * len(out_names),
                                check_rep=False)
        jitted = jax.jit(smapped, keep_unused=True)
        sh = NamedSharding(mesh, PartitionSpec("core"))
        zconcat = [
            jax.device_put(
                np.zeros((N_CORES * z.shape[0], *z.shape[1:]), z.dtype), sh)
            for z in zero_outs]
        _RUNNER_CACHE[key] = (jitted, in_names, out_names, out_avals, sh, zconcat)
    import jax
    jitted, in_names, out_names, out_avals, sh, zconcat = _RUNNER_CACHE[key]
    args = [
        jax.device_put(
            np.concatenate([in_maps[c][nm] for c in range(N_CORES)], axis=0), sh)
        for nm in in_names]
    outs = jitted(*args, *zconcat)
    oi = out_names.index("out")
    full = np.asarray(outs[oi]).reshape(N_CORES, *out_avals[oi].shape)
    return [full[c] for c in range(N_CORES)]


# revision 47
# speedup vs baseline: 1.9644x; 1.1392x over previous
"""Trainium2 Bass kernel for ConvexContractionAttention.

Math notes (derived from the reference):
  block(xi, w, b, a, g, beta) with h = xi*softplus(w)+b, h' = h @ qr(a).Q,
  then batch-norm over (B,T) per (d,j) feature reduces to an affine map of
  the centered input channel:
      out[b,t,d,j] = (xi[b,t,d] - mean_d(xi)) * A[d,j] + beta[d,j]
      A[d,j] = u[d,j]*g[d,j] / sqrt(var_d(xi)*u[d,j]^2 + eps_norm)
      u = softplus(w) @ Q          (bias b cancels through the mean)
  With beta == 0 (true for setup_inputs), per channel d:
      p    = xcq * xck
      s_j  = sigmoid(gamma*Aq_j*Ak_j * p)
      out0 = xcv * (sum_j s_j*Av_j) / (sum_j s_j + eps_w)
  followed by one more batch-affine-norm over (B,T) per channel.

Sharding: channel dim d=1024 split 128-per-core across 8 cores (fully
independent per channel; no collectives). On-chip layout: channels on the
128 SBUF partitions, B*T=8192 on the free axis; the host pre-transposes
each core's shard so every DMA is contiguous.

v2 pipeline (this file):
  - x staged in fp16 (halves HBM traffic; ~0.05% quantization, well under
    the 2e-2 gate), output stored fp16 and cast to fp32 on host.
  - per-body engine split: ScalarE = 3 sigmoids + Square-accum reductions;
    DVE = bn_stats(q), cheap 4x tensor_scalar work, fp32 recip + PSUM
    reads; PE = den/num j-sums (diag matmuls) incl. the eps term;
    GPSIMD = p product and the final out STT.
  - all tile pools are created once with bufs=2 and rotate per body, so
    consecutive bodies pipeline (body i+1's DMAs overlap body i compute).
"""

import sys

if "/opt/trn_rl_repo" not in sys.path:
    sys.path.insert(0, "/opt/trn_rl_repo")

import contextlib

import numpy as np

import concourse.bacc as bacc
import concourse.tile as tile
from concourse import mybir
from concourse import bass_utils

B, T, D = 4, 2048, 1024
BT = B * T
N_CORES = 8
DL = D // N_CORES  # 128 channels per core == SBUF partitions
GAMMA = 5.0
EPS_NORM = 1e-5
EPS_W = 1e-8

F32 = mybir.dt.float32
F16 = mybir.dt.float16
BF16 = mybir.dt.bfloat16
F32R = mybir.dt.float32r
F8 = mybir.dt.float8e4
Act = mybir.ActivationFunctionType
Alu = mybir.AluOpType

FB = 512          # PSUM bank block
HALF = BT // 2    # 4096
QTR = BT // 4     # 2048

# Engine assignment knobs (tuned empirically).
P_ON_GPSIMD = False
OUT_ON_GPSIMD = False
XC_ON_GPSIMD = False


def _emit_rsqrt(nc, pool, v, n, tag):
    """out = 1/sqrt(v) elementwise on a tiny [DL, n] fp32 tile, DVE-only.

    Bit-trick seed + 3 Newton iterations (~1e-7 rel); avoids the ScalarE
    Sqrt table set so the whole kernel stays on the sigmoid set.
    """
    U32 = mybir.dt.uint32
    bitsf = pool.tile([DL, n], F32, name=f"rsq_b_{tag}", tag=f"rsq_b_{tag}")
    nc.vector.tensor_copy(bitsf, v.bitcast(U32))
    nc.vector.tensor_scalar(
        out=bitsf, in0=bitsf, scalar1=-0.5, scalar2=1597463007.0,
        op0=Alu.mult, op1=Alu.add,
    )
    yu = pool.tile([DL, n], U32, name=f"rsq_y_{tag}", tag=f"rsq_y_{tag}")
    nc.vector.tensor_copy(yu, bitsf)
    y = yu.bitcast(F32)
    t = pool.tile([DL, n], F32, name=f"rsq_t_{tag}", tag=f"rsq_t_{tag}")
    for _ in range(3):
        nc.vector.tensor_mul(t, y, y)          # y^2
        nc.vector.tensor_mul(t, t, v)          # v*y^2
        nc.vector.tensor_scalar(
            out=t, in0=t, scalar1=-0.5, scalar2=1.5, op0=Alu.mult, op1=Alu.add,
        )                                      # 1.5 - 0.5*v*y^2
        nc.vector.tensor_mul(y, y, t)
    return y


def _emit_loads(nc, dram, pools, glob_tiles):
    """DMA issues only. Returns context for _emit_stats/_emit_main."""
    res, consts, temps, stage, psum = pools
    ident_sb, epsmat, ones_r = glob_tiles
    chunks = ("q", "k", "v")

    # ---- one packed parameter DMA ----
    pp = consts.tile([DL, 20], F32, name="pp", tag="pp")
    nc.sync.dma_start(out=pp, in_=dram["pp"])
    u_sb = {"q": pp[:, 0:3], "k": pp[:, 6:9], "v": pp[:, 12:15]}
    ug_sb = {"q": pp[:, 3:6], "k": pp[:, 9:12], "v": pp[:, 15:18]}
    g_out_sb = pp[:, 18:19]
    b_out_sb = pp[:, 19:20]

    # ---- x DMAs (fp16 compute copy + fp8 transposed stats copy) ----
    x_sb, xt_sb = {}, {}
    for p in chunks:
        x_sb[p] = res.tile([DL, BT], F16, name=f"x_{p}", tag=f"x_{p}")
        nc.sync.dma_start(out=x_sb[p], in_=dram["x" + p])
        xt_sb[p] = []
        for i in range(2):
            xh = res.tile([DL, HALF], F8, name=f"xt_{p}{i}", tag="xt", bufs=2)
            nc.sync.dma_start(out=xh, in_=dram["xt" + p][:, i * HALF:(i + 1) * HALF])
            xt_sb[p].append(xh)

    return dict(x_sb=x_sb, xt_sb=xt_sb, pp=pp, u_sb=u_sb, ug_sb=ug_sb,
                g_out_sb=g_out_sb, b_out_sb=b_out_sb)


def _emit_stats(nc, dram, pools, glob_tiles, c):
    """Stats + per-body constants (emitted after the previous body's main
    loop so the PE/DVE streams never block on the next body's inputs)."""
    res, consts, temps, stage, psum = pools
    ident_sb, epsmat, ones_r = glob_tiles
    chunks = ("q", "k", "v")
    x_sb, xt_sb = c["x_sb"], c["xt_sb"]
    u_sb, ug_sb = c["u_sb"], c["ug_sb"]

    # sum(x) per chunk via DVE tensor_scalar accum (4x mode)
    sums = {}
    for p in chunks:
        sums[p] = consts.tile([DL, 4], F32, name=f"sum_{p}", tag=f"sum_{p}")
        for h in range(4):
            sl = slice(h * QTR, (h + 1) * QTR)
            scr16 = temps.tile([DL, QTR], F16, name="scp", tag="xcq", bufs=2)
            nc.vector.tensor_scalar(
                out=scr16, in0=x_sb[p][:, sl], scalar1=1.0, scalar2=0.0,
                op0=Alu.mult, op1=Alu.add, accum_out=sums[p][:, h:h + 1],
            )
    # sum(x^2) per chunk via PE Gram on the fp8 transposed copy: G = sum_i
    # Xi^T Xi accumulated in PSUM; diag(G)[d] = sum_t x[d,t]^2.
    sqs = {}
    for p in chunks:
        gfull = psum.tile([DL, QTR], F32, name=f"g_{p}", tag="pden", bufs=1)
        gps = gfull[:, 0:DL]
        for i in range(64):
            xt_i = xt_sb[p][i // 32][:, (i % 32) * DL:(i % 32 + 1) * DL]
            nc.tensor.matmul(out=gps, lhsT=xt_i, rhs=xt_i,
                             start=(i == 0), stop=(i == 63))
        gd = temps.tile([DL, DL], F32, name="gd", tag="gd")
        nc.vector.tensor_mul(gd, gps, ident_sb)
        sqs[p] = consts.tile([DL, 1], F32, name=f"sq_{p}", tag=f"sq_{p}")
        nc.vector.tensor_reduce(sqs[p], gd, axis=mybir.AxisListType.X, op=Alu.add)

    # per-chunk mean/var -> A matrices
    mv = {}
    for p in chunks:
        mvp = consts.tile([DL, 2], F32, name=f"mv_{p}", tag=f"mv_{p}")
        ssum = consts.tile([DL, 1], F32, name=f"ss_{p}", tag=f"ss_{p}")
        nc.vector.tensor_reduce(ssum, sums[p], axis=mybir.AxisListType.X, op=Alu.add)
        nc.vector.tensor_scalar_mul(out=mvp[:, 0:1], in0=ssum, scalar1=1.0 / BT)
        msq = consts.tile([DL, 1], F32, name=f"msq_{p}", tag=f"msq_{p}")
        nc.vector.tensor_mul(msq, mvp[:, 0:1], mvp[:, 0:1])
        nc.vector.scalar_tensor_tensor(
            out=mvp[:, 1:2], in0=sqs[p], scalar=1.0 / BT, in1=msq,
            op0=Alu.mult, op1=Alu.subtract,
        )
        mv[p] = mvp

    vterm = consts.tile([DL, 9], F32, name="vt", tag="vt")
    for pi, p in enumerate(chunks):
        usq = consts.tile([DL, 3], F32, name=f"usq_{p}", tag=f"usq_{p}")
        nc.vector.tensor_mul(usq, u_sb[p], u_sb[p])
        nc.vector.tensor_scalar(
            out=vterm[:, 3 * pi:3 * pi + 3], in0=usq, scalar1=mv[p][:, 1:2],
            scalar2=EPS_NORM, op0=Alu.mult, op1=Alu.add,
        )
    inv9 = _emit_rsqrt(nc, consts, vterm, 9, "A")
    Amat = {}
    for pi, p in enumerate(chunks):
        Amat[p] = consts.tile([DL, 3], F32, name=f"A_{p}", tag=f"A_{p}")
        nc.vector.tensor_mul(Amat[p], ug_sb[p], inv9[:, 3 * pi:3 * pi + 3])
    cmat = consts.tile([DL, 3], F32, name="cmat", tag="cmat")
    nc.vector.tensor_mul(cmat, Amat["q"], Amat["k"])
    nc.vector.tensor_scalar_mul(out=cmat, in0=cmat, scalar1=GAMMA)

    muq = mv["q"][:, 0:1]
    muk = mv["k"][:, 0:1]
    muv = mv["v"][:, 0:1]
    Av = Amat["v"]

    # diag(Av_j) in f32r for the num matmuls (f32r keeps the sigmoid tail —
    # fp16 flushes sigma~1e-8..1e-35 to 0 but the reference's tiny-sigma
    # ratios still produce O(1) weights there)
    dg = []
    for j in range(3):
        d = consts.tile([DL, DL], BF16, name=f"dg{j}", tag=f"dg{j}")
        nc.vector.tensor_scalar_mul(out=d, in0=ident_sb, scalar1=Av[:, j:j + 1])
        dg.append(d)

    c.update(muq=muq, muk=muk, muv=muv, cmat=cmat, dg=dg)
    return c


def _emit_main(nc, dram, pools, glob_tiles, c):
    """Main loop + out-stats + final affine + store for a prepared body."""
    res, consts, temps, stage, psum = pools
    ident_sb, epsmat, ones_r = glob_tiles
    x_sb = c["x_sb"]
    muq, muk, muv = c["muq"], c["muk"], c["muv"]
    cmat, dg = c["cmat"], c["dg"]

    out_sb = res.tile([DL, BT], F16, name="out", tag="out", bufs=1)
    osum = consts.tile([DL, 4], F32, name="osum", tag="osum")
    osq = consts.tile([DL, 4], F32, name="osq", tag="osq")

    # xc/p for all quarters upfront: gives GPSIMD and ScalarE lookahead
    pprods = []
    for qt in range(4):
        qsl = slice(qt * QTR, (qt + 1) * QTR)
        xcq = temps.tile([DL, QTR], F16, name="xcq", tag="xcq", bufs=2)
        xck = temps.tile([DL, QTR], F16, name="xck", tag="xck", bufs=2)
        pprod = temps.tile([DL, QTR], F16, name="pprod", tag="pprod", bufs=2)
        nc.vector.tensor_scalar(
            out=xcq, in0=x_sb["q"][:, qsl], scalar1=muq, scalar2=None,
            op0=Alu.subtract,
        )
        nc.vector.tensor_scalar(
            out=xck, in0=x_sb["k"][:, qsl], scalar1=muk, scalar2=None,
            op0=Alu.subtract,
        )
        eng = nc.gpsimd if P_ON_GPSIMD else nc.vector
        eng.tensor_mul(pprod, xcq, xck)  # fp16 TT: DVE 2x or Pool
        pprods.append(pprod)

    for qt in range(4):
        qsl = slice(qt * QTR, (qt + 1) * QTR)
        # sigmoids (bf16 out for the PE)
        sig = []
        for j in range(3):
            s = temps.tile([DL, QTR], BF16, name=f"s{j}", tag=f"s{j}", bufs=2)
            nc.scalar.activation(s, pprods[qt], Act.Sigmoid,
                                 scale=cmat[:, j:j + 1])
            sig.append(s)
        # one 2048-wide block per quarter: den/num on PE (7 matmuls, each
        # lhsT loaded once), recip+tt on DVE, out product on GPSIMD
        pden = psum.tile([DL, QTR], F32, name="pden", tag="pden", bufs=1)
        pnum = psum.tile([DL, QTR], F32, name="pnum", tag="pnum", bufs=1)
        for b4 in range(QTR // FB):
            ps = slice(b4 * FB, (b4 + 1) * FB)
            nc.tensor.matmul(out=pden[:, ps], lhsT=epsmat, rhs=ones_r[:, ps],
                             start=True, stop=False)
        for j in range(3):
            for b4 in range(QTR // FB):
                ps = slice(b4 * FB, (b4 + 1) * FB)
                nc.tensor.matmul(out=pden[:, ps], lhsT=ident_sb,
                                 rhs=sig[j][:, ps],
                                 start=False, stop=(j == 2))
        for j in range(3):
            for b4 in range(QTR // FB):
                ps = slice(b4 * FB, (b4 + 1) * FB)
                nc.tensor.matmul(out=pnum[:, ps], lhsT=dg[j],
                                 rhs=sig[j][:, ps],
                                 start=(j == 0), stop=(j == 2))
        rr = temps.tile([DL, QTR], F32, name="rr", tag="rr")
        nc.vector.reciprocal_approx_fast(out=rr, in_=pden)
        ttb = temps.tile([DL, QTR], F16, name="ttb", tag="ttb", bufs=1)
        nc.vector.tensor_mul(ttb, pnum, rr)
        nc.vector.scalar_tensor_tensor(
            out=out_sb[:, qsl], in0=x_sb["v"][:, qsl], scalar=muv, in1=ttb,
            op0=Alu.subtract, op1=Alu.mult, accum_out=osum[:, qt:qt + 1],
        )
    for qt in range(4):
        qsl = slice(qt * QTR, (qt + 1) * QTR)
        scrb2 = temps.tile([DL, QTR], BF16, name="scb", tag="scb")
        nc.scalar.activation(scrb2, out_sb[:, qsl], Act.Square,
                             accum_out=osq[:, qt:qt + 1])


    # ---- final norm constants ----
    sum_o = consts.tile([DL, 1], F32, name="sum_o", tag="sum_o")
    nc.vector.tensor_reduce(sum_o, osum, axis=mybir.AxisListType.X, op=Alu.add)
    sq_o = consts.tile([DL, 1], F32, name="sq_o", tag="sq_o")
    nc.vector.tensor_reduce(sq_o, osq, axis=mybir.AxisListType.X, op=Alu.add)
    mean_o = consts.tile([DL, 1], F32, name="mean_o", tag="mean_o")
    nc.vector.tensor_scalar_mul(out=mean_o, in0=sum_o, scalar1=1.0 / BT)
    msq_o = consts.tile([DL, 1], F32, name="msq_o", tag="msq_o")
    nc.vector.tensor_mul(msq_o, mean_o, mean_o)
    var_o = consts.tile([DL, 1], F32, name="var_o", tag="var_o")
    nc.vector.scalar_tensor_tensor(
        out=var_o, in0=sq_o, scalar=1.0 / BT, in1=msq_o,
        op0=Alu.mult, op1=Alu.subtract,
    )
    nc.vector.tensor_scalar_add(out=var_o, in0=var_o, scalar1=EPS_NORM)
    rs_o = _emit_rsqrt(nc, consts, var_o, 1, "o")
    fs = consts.tile([DL, 1], F32, name="fs", tag="fs")
    nc.vector.tensor_mul(fs, c["g_out_sb"], rs_o)
    fbt = consts.tile([DL, 1], F32, name="fbt", tag="fbt")
    nc.vector.tensor_mul(fbt, mean_o, fs)
    fb = consts.tile([DL, 1], F32, name="fb", tag="fb")
    nc.vector.tensor_sub(fb, c["b_out_sb"], fbt)

    # ---- final affine + store (fp16, GPSIMD so DVE is free for the next
    # body's stats) ----
    for i in range(4):
        sl = slice(i * QTR, (i + 1) * QTR)
        stg = temps.tile([DL, QTR], F16, name="stg", tag="xcv", bufs=2)
        nc.scalar.activation(stg, out_sb[:, sl], Act.Identity,
                             bias=fb, scale=fs)
        nc.gpsimd.dma_start(out=dram["out"][:, sl], in_=stg)


def build_program(reps=1, variant="v2"):
    nc = bacc.Bacc("TRN2", num_devices=N_CORES)
    dram = {}
    for p in ("q", "k", "v"):
        dram["x" + p] = nc.dram_tensor("x" + p, [DL, BT], F16, kind="ExternalInput").ap()
        dram["xt" + p] = nc.dram_tensor("xt" + p, [DL, BT], F8, kind="ExternalInput").ap()
    dram["pp"] = nc.dram_tensor("pp", [DL, 20], F32, kind="ExternalInput").ap()
    dram["ident"] = nc.dram_tensor("ident", [DL, DL], BF16, kind="ExternalInput").ap()
    dram["out"] = nc.dram_tensor("out", [DL, BT], F16, kind="ExternalOutput").ap()

    with tile.TileContext(nc) as tc:
        with contextlib.ExitStack() as ctx:
            glob = ctx.enter_context(tc.tile_pool(name="glob", bufs=1))
            res = ctx.enter_context(tc.tile_pool(name="res", bufs=2))
            consts = ctx.enter_context(tc.tile_pool(name="consts", bufs=2))
            temps = ctx.enter_context(tc.tile_pool(name="temps", bufs=1))
            stage = ctx.enter_context(tc.tile_pool(name="stage", bufs=2))
            psum = ctx.enter_context(tc.tile_pool(name="psum", bufs=2, space="PSUM"))

            # global constants, loaded once
            ident_sb = glob.tile([DL, DL], BF16, name="ident", tag="ident")
            nc.sync.dma_start(out=ident_sb, in_=dram["ident"])
            epsmat = glob.tile([DL, DL], BF16, name="epsmat", tag="epsmat")
            nc.vector.memset(epsmat, EPS_W / DL)
            ones_r = glob.tile([DL, QTR], BF16, name="ones_r", tag="ones_r")
            nc.vector.memset(ones_r, 1.0)
            warm = glob.tile([DL, 1], F32, name="warm", tag="warm")
            nc.vector.memset(warm, 0.0)
            nc.scalar.activation(warm, warm, Act.Sigmoid)

            glob_tiles = (ident_sb, epsmat, ones_r)
            pools = (res, consts, temps, stage, psum)
            # software-pipelined emission: body i+1's loads are issued
            # before body i's main loop (DMA overlap), and body i+1's
            # stats/consts are emitted after it (so the PE stream runs
            # main(i) matmuls before Gram(i+1), and no engine blocks on a
            # body tail while independent next-body work waits).
            cur = _emit_loads(nc, dram, pools, glob_tiles)
            cur = _emit_stats(nc, dram, pools, glob_tiles, cur)
            for i in range(reps):
                nxt = None
                if i + 1 < reps:
                    nxt = _emit_loads(nc, dram, pools, glob_tiles)
                _emit_main(nc, dram, pools, glob_tiles, cur)
                if nxt is not None:
                    cur = _emit_stats(nc, dram, pools, glob_tiles, nxt)
    nc.compile()
    return nc


def _softplus(x):
    return np.log1p(np.exp(-np.abs(x))) + np.maximum(x, 0.0)


def _host_params(w, b, a, g, beta):
    """Return (u, u*g) per channel (bias b cancels through the mean)."""
    Q = np.linalg.qr(np.asarray(a, dtype=np.float64))[0].astype(np.float32)
    u = np.einsum("di,dij->dj", _softplus(np.asarray(w, np.float64)).astype(np.float32), Q)
    return u, u * np.asarray(g, np.float32)


def _reference_fallback(x, wq, bq, aq, gq, betaq, wk, bk, ak, gk, betak,
                        wv, bv, av, gv, betav, g_out, b_out):
    """General-path numpy fallback (only used if some beta is nonzero)."""
    def block(xi, w, b, a, g, beta):
        h = xi[..., None] * _softplus(w) + b
        Q = np.linalg.qr(a)[0]
        h = np.einsum("btdi,dij->btdj", h, Q)
        mean = h.mean(axis=(0, 1))
        var = h.var(axis=(0, 1))
        return (h - mean) / np.sqrt(var + EPS_NORM) * g + beta

    d = D
    Qp = block(x[..., :d], wq, bq, aq, gq, betaq)
    Kp = block(x[..., d:2 * d], wk, bk, ak, gk, betak)
    Vp = block(x[..., 2 * d:], wv, bv, av, gv, betav)
    scores = 1.0 / (1.0 + np.exp(-GAMMA * (Qp * Kp)))
    weights = scores / (scores.sum(axis=-1, keepdims=True) + EPS_W)
    out = (weights * Vp).sum(axis=-1)
    mean = out.mean(axis=(0, 1))
    var = out.var(axis=(0, 1))
    return ((out - mean) / np.sqrt(var + EPS_NORM) * g_out + b_out).astype(np.float32)


_NC_CACHE = {}

VARIANT = "v2"


def _get_program(reps=1, variant=None):
    if variant is None:
        variant = VARIANT
    key = (reps, variant)
    if key not in _NC_CACHE:
        _NC_CACHE[key] = build_program(reps, variant)
    return _NC_CACHE[key]


def _make_in_maps(x, params):
    """params: dict p -> (u, ug) full (D,3); x: (B,T,3D). Returns per-core maps."""
    x2 = np.asarray(x, np.float32).reshape(BT, 3 * D)
    # one-pass transpose into (24 blocks, DL channels, BT) channel-major, fp16
    xt = np.ascontiguousarray(
        x2.reshape(BT, 3 * N_CORES, DL).transpose(1, 2, 0)).astype(np.float16)
    in_maps = []
    for c in range(N_CORES):
        m = {}
        pp = np.empty((DL, 20), np.float32)
        import ml_dtypes
        for pi, p in enumerate(("q", "k", "v")):
            xc = xt[pi * N_CORES + c]
            m["x" + p] = xc
            m["xt" + p] = np.ascontiguousarray(
                xc.reshape(DL, BT // DL, DL).transpose(2, 1, 0).reshape(DL, BT)
            ).astype(ml_dtypes.float8_e4m3)
            u, ug = params[p]
            pp[:, 6 * pi:6 * pi + 3] = u[c * DL:(c + 1) * DL]
            pp[:, 6 * pi + 3:6 * pi + 6] = ug[c * DL:(c + 1) * DL]
        pp[:, 18] = params["g_out"][c * DL:(c + 1) * DL]
        pp[:, 19] = params["b_out"][c * DL:(c + 1) * DL]
        m["pp"] = pp
        import ml_dtypes
        m["ident"] = np.eye(DL, dtype=ml_dtypes.bfloat16)
        in_maps.append(m)
    return in_maps


def kernel(x, wq, bq, aq, gq, betaq, wk, bk, ak, gk, betak,
           wv, bv, av, gv, betav, g_out, b_out):
    if (np.any(np.asarray(betaq)) or np.any(np.asarray(betak))
            or np.any(np.asarray(betav))):
        return _reference_fallback(x, wq, bq, aq, gq, betaq, wk, bk, ak, gk,
                                   betak, wv, bv, av, gv, betav, g_out, b_out)

    params = {
        "q": _host_params(wq, bq, aq, gq, betaq),
        "k": _host_params(wk, bk, ak, gk, betak),
        "v": _host_params(wv, bv, av, gv, betav),
        "g_out": np.asarray(g_out, np.float32),
        "b_out": np.asarray(b_out, np.float32),
    }
    nc = _get_program()
    in_maps = _make_in_maps(x, params)
    try:
        per_core = _run_cached(nc, in_maps)
    except Exception:
        res = bass_utils.run_bass_kernel_spmd(
            nc, in_maps, core_ids=list(range(N_CORES)))
        per_core = [res.results[c]["out"] for c in range(N_CORES)]
    out = np.empty((BT, D), np.float32)
    for c in range(N_CORES):
        out[:, c * DL:(c + 1) * DL] = np.asarray(per_core[c], np.float32).T
    return out.reshape(B, T, D)


_RUNNER_CACHE = {}


def _run_cached(nc, in_maps):
    """Jit the bass_exec shard_map once; later kernel() calls only restage
    inputs (saves ~1-2 s of retracing/recompiling per call)."""
    key = id(nc)
    if key not in _RUNNER_CACHE:
        import jax
        from jax.sharding import Mesh, PartitionSpec, NamedSharding
        try:
            from jax import shard_map
        except ImportError:
            from jax.experimental.shard_map import shard_map
        from concourse import mybir as _mb
        from concourse.bass2jax import (
            _bass_exec_p, install_neuronx_cc_hook, partition_id_tensor)

        install_neuronx_cc_hook()
        pname = nc.partition_id_tensor.name if nc.partition_id_tensor else None
        in_names, out_names, out_avals, zero_outs = [], [], [], []
        for alloc in nc.m.functions[0].allocations:
            if not isinstance(alloc, _mb.MemoryLocationSet):
                continue
            name = alloc.memorylocations[0].name
            if alloc.kind == "ExternalInput":
                if name != pname:
                    in_names.append(name)
            elif alloc.kind == "ExternalOutput":
                out_names.append(name)
                shp = tuple(alloc.tensor_shape)
                dt_np = _mb.dt.np(alloc.dtype)
                out_avals.append(jax.core.ShapedArray(shp, dt_np))
                zero_outs.append(np.zeros(shp, dt_np))
        all_in = list(in_names) + list(out_names)
        if pname is not None:
            all_in.append(pname)

        def _body(*args):
            operands = list(args)
            if pname is not None:
                operands.append(partition_id_tensor())
            return tuple(_bass_exec_p.bind(
                *operands, out_avals=tuple(out_avals), in_names=tuple(all_in),
                out_names=tuple(out_names), lowering_input_output_aliases=(),
                sim_require_finite=True, sim_require_nnan=True, nc=nc))

        devices = jax.devices()[:N_CORES]
        mesh = Mesh(np.asarray(devices), ("core",))
        nspec = (PartitionSpec("core"),) * (len(in_names) + len(out_names))
        try:
            smapped = shard_map(_body, mesh=mesh, in_specs=nspec,
                                out_specs=(PartitionSpec("core"),) * len(out_names),
                                check_vma=False)
        except TypeError:
            smapped = shard_map(_body, mesh=mesh, in_specs=nspec,
                                out_specs=(PartitionSpec("core"),) * len(out_names),
                                check_rep=False)
        jitted = jax.jit(smapped, keep_unused=True)
        sh = NamedSharding(mesh, PartitionSpec("core"))
        zconcat = [
            jax.device_put(
                np.zeros((N_CORES * z.shape[0], *z.shape[1:]), z.dtype), sh)
            for z in zero_outs]
        _RUNNER_CACHE[key] = (jitted, in_names, out_names, out_avals, sh, zconcat)
    import jax
    jitted, in_names, out_names, out_avals, sh, zconcat = _RUNNER_CACHE[key]
    args = [
        jax.device_put(
            np.concatenate([in_maps[c][nm] for c in range(N_CORES)], axis=0), sh)
        for nm in in_names]
    outs = jitted(*args, *zconcat)
    oi = out_names.index("out")
    full = np.asarray(outs[oi]).reshape(N_CORES, *out_avals[oi].shape)
    return [full[c] for c in range(N_CORES)]
